# revision 1
# baseline (speedup 1.0000x reference)
"""Trainium2 Bass kernel for 12-head causal MHA (B=4, S=2048, D=768).

Sharding: 8 cores, core c -> (batch c//2, query-row parity c%2).
Each core computes the full attention output for query rows
g = 2*t + parity of its batch (1024 rows), which makes the causal loop
structure identical on every core (single SPMD Bass program) and the
gather a pure row-interleave.

Layout is fully transposed so every matmul contracts along partitions:
  qT/kT: [head_dim, seq]  scoresT: [sk, sq]  ctxT': [hd+1, sq]
The softmax row-sum is fused into the ctx matmul via a ones column
appended to V (M=65).  Softmax skips max-subtraction (scores/8 are
bounded by ~2 for this distribution, exp is safe).
"""

import os
import sys
from contextlib import ExitStack

import numpy as np

os.environ.setdefault("MYCRO_LOCAL_CACHE", "1")

for _p in ("/root/.axon_site/_ro/trn_rl_repo", "/opt/trn_rl_repo"):
    # later inserts win: prefer /opt (writable sibling modules, e.g.
    # antenv.axon_hooks) over the read-only mirror
    if os.path.isdir(_p) and _p not in sys.path:
        sys.path.insert(0, _p)

import concourse.bass as bass  # noqa: E402
import concourse.tile as tile  # noqa: E402
from concourse import bacc, mybir  # noqa: E402
from concourse.bass_utils import run_bass_kernel_spmd  # noqa: E402

B, S, D, H, HD = 4, 2048, 768, 12, 64
NPAIR = H // 2          # 6 head pairs (2 heads packed per 128 partitions)
SQL = S // 2            # 1024 local query rows per core
JB = SQL // 256         # 4 local 256-col blocks
KC = S // 128           # 16 key chunks
DC = D // 128           # 6 contraction chunks for the projections
N_CORES = 8

F32 = mybir.dt.float32
F32R = mybir.dt.float32r
BF16 = mybir.dt.bfloat16
EXP = mybir.ActivationFunctionType.Exp

LAST_RESULT = None  # BassKernelResults of the most recent run (for test.py)

_CACHED_NC = None


def _r(ap):
    """f32r matmul operand (tiles on these paths are float32r-typed)."""
    return ap


def build_nc():
    nc = bacc.Bacc("TRN2", target_bir_lowering=False)

    xT = nc.dram_tensor("xT", [D, S], BF16, kind="ExternalInput")
    xTq = nc.dram_tensor("xTq", [D, SQL], BF16, kind="ExternalInput")
    wqT = nc.dram_tensor("wqT", [D, D], BF16, kind="ExternalInput")
    wkT = nc.dram_tensor("wkT", [D, D], BF16, kind="ExternalInput")
    wvT = nc.dram_tensor("wvT", [D, D], BF16, kind="ExternalInput")
    woT = nc.dram_tensor("woT", [D, D], BF16, kind="ExternalInput")
    masks = nc.dram_tensor("masks", [4, 128, 256], BF16, kind="ExternalInput")
    bo_d = nc.dram_tensor("bo", [1, D], F32, kind="ExternalInput")
    out_d = nc.dram_tensor("out", [SQL, D], F32, kind="ExternalOutput")

    with tile.TileContext(nc) as tc, ExitStack() as ctx:
        pers = ctx.enter_context(tc.tile_pool(name="pers", bufs=1))
        kT6 = pers.tile([128, NPAIR, S], BF16)          # kT, pair-stacked
        v3 = pers.tile([128, KC, H, HD + 1], BF16)      # v (+ones col) per chunk
        qT6 = pers.tile([128, NPAIR, SQL], BF16)
        ctx6 = pers.tile([128, NPAIR, SQL], BF16)       # normalized ctxT
        ones_sb = pers.tile([65, 128], F32)
        mask_sb = pers.tile([128, 4, 256], BF16)
        bo_sb = pers.tile([128, D], F32)

        nc.vector.memset(ones_sb, 1.0)
        nc.vector.memset(v3[:, :, :, HD], 1.0)         # ones cols, stride 65
        for mi in range(4):
            nc.sync.dma_start(out=mask_sb[:, mi, :], in_=masks[mi])

        # --- broadcast bo across partitions once (rank-1 matmul trick) ---
        with (
            tc.tile_pool(name="pre_s", bufs=1) as pre_s,
            tc.tile_pool(name="pre_p", bufs=1, space="PSUM") as pre_p,
        ):
            bo_row = pre_s.tile([1, D], F32)
            nc.sync.dma_start(out=bo_row, in_=bo_d[:])
            pbo = pre_p.tile([128, D], F32)
            for lo, hi in ((0, 512), (512, D)):
                nc.tensor.matmul(pbo[:, lo:hi], lhsT=ones_sb[0:1, :],
                                 rhs=bo_row[0:1, lo:hi], start=True, stop=True)
            nc.vector.tensor_copy(bo_sb, pbo)

        # --- projections: K, V, then Q (weights staged one at a time) ---
        with (
            tc.tile_pool(name="wstage", bufs=3) as wpool,
            tc.tile_pool(name="xstage", bufs=3) as xpool,
            tc.tile_pool(name="pproj", bufs=3, space="PSUM") as ppool,
        ):
            # K projection: kT6[:, r, s] for all 2048 keys
            wk = wpool.tile([128, DC, D], BF16, tag="w")
            for k in range(DC):
                nc.sync.dma_start(out=wk[:, k, :], in_=wkT[128 * k:128 * (k + 1), :])
            for sb in range(S // 512):
                xk = xpool.tile([128, DC, 512], BF16, tag="x")
                for k in range(DC):
                    nc.sync.dma_start(
                        out=xk[:, k, :],
                        in_=xT[128 * k:128 * (k + 1), 512 * sb:512 * (sb + 1)])
                for r in range(NPAIR):
                    ps = ppool.tile([128, 512], F32, tag="pk")
                    for k in range(DC):
                        nc.tensor.matmul(
                            ps, lhsT=_r(wk[:, k, 128 * r:128 * (r + 1)]),
                            rhs=_r(xk[:, k, :]),
                            start=(k == 0), stop=(k == DC - 1))
                    nc.vector.tensor_copy(kT6[:, r, 512 * sb:512 * (sb + 1)], ps)

            # V projection: v3[:, a, h, 0:64] per 128-key chunk a
            wv = wpool.tile([128, DC, D], BF16, tag="w")
            for k in range(DC):
                nc.sync.dma_start(out=wv[:, k, :], in_=wvT[128 * k:128 * (k + 1), :])
            for a in range(KC):
                xa = xpool.tile([128, DC, 128], BF16, tag="x")
                for k in range(DC):
                    nc.sync.dma_start(
                        out=xa[:, k, :],
                        in_=xT[128 * k:128 * (k + 1), 128 * a:128 * (a + 1)])
                ps = ppool.tile([128, D], F32, tag="pk")
                for lo, hi in ((0, 512), (512, D)):
                    for k in range(DC):
                        nc.tensor.matmul(
                            ps[:, lo:hi], lhsT=_r(xa[:, k, :]),
                            rhs=_r(wv[:, k, lo:hi]),
                            start=(k == 0), stop=(k == DC - 1))
                nc.vector.tensor_copy(
                    v3[:, a, :, 0:HD],
                    ps.rearrange("p (h e) -> p h e", e=HD))

            # Q projection (only this core's 1024 query rows)
            wq = wpool.tile([128, DC, D], BF16, tag="w")
            for k in range(DC):
                nc.sync.dma_start(out=wq[:, k, :], in_=wqT[128 * k:128 * (k + 1), :])
            for j2 in range(SQL // 512):
                xq = xpool.tile([128, DC, 512], BF16, tag="x")
                for k in range(DC):
                    nc.sync.dma_start(
                        out=xq[:, k, :],
                        in_=xTq[128 * k:128 * (k + 1), 512 * j2:512 * (j2 + 1)])
                for r in range(NPAIR):
                    ps = ppool.tile([128, 512], F32, tag="pk")
                    for k in range(DC):
                        nc.tensor.matmul(
                            ps, lhsT=_r(wq[:, k, 128 * r:128 * (r + 1)]),
                            rhs=_r(xq[:, k, :]),
                            start=(k == 0), stop=(k == DC - 1))
                    nc.vector.tensor_copy(qT6[:, r, 512 * j2:512 * (j2 + 1)], ps)

        # --- attention ---
        with (
            tc.tile_pool(name="spool", bufs=2, space="PSUM") as spool,
            tc.tile_pool(name="cpool", bufs=3, space="PSUM") as cpool,
            tc.tile_pool(name="bpool", bufs=1, space="PSUM") as bpool,
            tc.tile_pool(name="epool", bufs=3) as epool,
            tc.tile_pool(name="rpool", bufs=2) as rpool,
        ):
            for r in range(NPAIR):
                for j in range(JB):
                    nch = 4 * j + 4
                    cA = cpool.tile([65, 256], F32, tag="c")
                    cB = cpool.tile([65, 256], F32, tag="c")
                    jsl = slice(256 * j, 256 * (j + 1))
                    for g in range(0, nch, 2):
                        # bank layout: [0:512) = head-A scores of sites g,g+1
                        # (bank 0); [512:1024) = head-B (bank 1).  Concurrent
                        # row-packed A/B matmuls never share a psum bank.
                        sp = spool.tile([128, 1024], F32, tag="s")
                        e = epool.tile([128, 1024], BF16, tag="e")
                        for si, a in enumerate((g, g + 1)):
                            bA = 256 * si
                            bB = 512 + 256 * si
                            asl = slice(128 * a, 128 * (a + 1))
                            # start=True clears the whole psum bank, so only
                            # the first matmul per bank may set it; the second
                            # writes fresh (has_written=0) elements with
                            # start=False and lands as an overwrite
                            nc.tensor.matmul(
                                sp[:, bA:bA + 256],
                                lhsT=_r(kT6[0:64, r, asl]),
                                rhs=_r(qT6[0:64, r, jsl]),
                                start=(si == 0), stop=True,
                                tile_position=(0, 0), skip_group_check=True)
                            nc.tensor.matmul(
                                sp[:, bB:bB + 256],
                                lhsT=_r(kT6[64:128, r, asl]),
                                rhs=_r(qT6[64:128, r, jsl]),
                                start=(si == 0), stop=True,
                                tile_position=(64, 0), skip_group_check=True)
                        nc.scalar.activation(e[:, 0:512], sp[:, 0:512],
                                             EXP, scale=0.125)
                        nc.scalar.activation(e[:, 512:1024], sp[:, 512:1024],
                                             EXP, scale=0.125)
                        for si, a in enumerate((g, g + 1)):
                            bA = 256 * si
                            bB = 512 + 256 * si
                            mi = a - 4 * j
                            z = 64 * mi if mi > 0 else 0
                            if mi >= 0:
                                ms = slice(64 * mi, 64 * mi + 64)
                                e_msA = slice(bA + 64 * mi, bA + 64 * mi + 64)
                                e_msB = slice(bB + 64 * mi, bB + 64 * mi + 64)
                                nc.vector.tensor_mul(
                                    e[:, e_msA], e[:, e_msA], mask_sb[:, mi, ms])
                                nc.vector.tensor_mul(
                                    e[:, e_msB], e[:, e_msB], mask_sb[:, mi, ms])
                            nc.tensor.matmul(
                                cA[:, z:256], lhsT=_r(v3[:, a, 2 * r, :]),
                                rhs=_r(e[:, bA + z:bA + 256]),
                                start=(a == 0), stop=(a == nch - 1))
                            nc.tensor.matmul(
                                cB[:, z:256], lhsT=_r(v3[:, a, 2 * r + 1, :]),
                                rhs=_r(e[:, bB + z:bB + 256]),
                                start=(a == 0), stop=(a == nch - 1))
                    # normalize: recip of fused row-sums, broadcast via PE
                    rr = rpool.tile([65, 512], F32, tag="rr")
                    nc.vector.reciprocal(rr[64:65, 0:256], cA[64:65, :])
                    nc.vector.reciprocal(rr[64:65, 256:512], cB[64:65, :])
                    pb = bpool.tile([128, 512], F32, tag="b")
                    nc.tensor.matmul(pb, lhsT=ones_sb[64:65, :],
                                     rhs=rr[64:65, :], start=True, stop=True)
                    pb_sb = rpool.tile([128, 512], F32, tag="pbs")
                    nc.vector.tensor_copy(pb_sb, pb)
                    nc.vector.tensor_mul(ctx6[0:64, r, jsl], cA[0:64, :],
                                         pb_sb[0:64, 0:256])
                    tB = rpool.tile([64, 256], BF16, tag="tB")
                    nc.vector.tensor_mul(tB, cB[0:64, :], pb_sb[0:64, 256:512])
                    # head B lands on partitions 64-127: remap via SBUF DMA
                    nc.sync.dma_start(out=ctx6[64:128, r, jsl], in_=tB)

        # --- output projection + bias ---
        with (
            tc.tile_pool(name="wopool", bufs=1) as wopool,
            tc.tile_pool(name="opool", bufs=2, space="PSUM") as opool,
            tc.tile_pool(name="ospool", bufs=3) as ospool,
        ):
            wo = wopool.tile([128, DC, D], BF16)
            for k in range(DC):
                nc.sync.dma_start(out=wo[:, k, :], in_=woT[128 * k:128 * (k + 1), :])
            for i in range(SQL // 128):
                isl = slice(128 * i, 128 * (i + 1))
                po = opool.tile([128, D], F32)
                for lo, hi in ((0, 512), (512, D)):
                    for r in range(NPAIR):
                        nc.tensor.matmul(
                            po[:, lo:hi], lhsT=_r(ctx6[:, r, isl]),
                            rhs=_r(wo[:, r, lo:hi]),
                            start=(r == 0), stop=(r == NPAIR - 1))
                osb = ospool.tile([128, D], F32)
                nc.vector.tensor_add(osb, po, bo_sb)
                nc.sync.dma_start(out=out_d[isl, :], in_=osb)

    nc.compile()
    return nc


def get_nc():
    global _CACHED_NC
    if _CACHED_NC is None:
        _CACHED_NC = build_nc()
    return _CACHED_NC


def make_core_inputs(x, wq, wk, wv, wo, bo):
    """Host-side shard prep: slices/transposes/dtype rounding only."""
    import ml_dtypes
    bf16 = ml_dtypes.bfloat16
    wqT = np.ascontiguousarray(wq.T.astype(bf16))
    wkT = np.ascontiguousarray(wk.T.astype(bf16))
    wvT = np.ascontiguousarray(wv.T.astype(bf16))
    woT = np.ascontiguousarray(wo.T.astype(bf16))
    bo_in = np.ascontiguousarray(bo.reshape(1, D))

    p_idx = np.arange(128)[:, None]
    u_idx = np.arange(256)[None, :]
    mask_by_half = []
    for half in range(2):
        m = np.zeros((4, 128, 256), ml_dtypes.bfloat16)
        for mi in range(4):
            m[mi] = (p_idx <= 2 * u_idx + half - 128 * mi)
        mask_by_half.append(m)

    in_maps = []
    for c in range(N_CORES):
        b, half = c // 2, c % 2
        xT_b = np.ascontiguousarray(x[b].T.astype(bf16))
        in_maps.append({
            "xT": xT_b,
            "xTq": np.ascontiguousarray(xT_b[:, half::2]),
            "wqT": wqT, "wkT": wkT, "wvT": wvT, "woT": woT,
            "masks": mask_by_half[half],
            "bo": bo_in,
        })
    return in_maps


def kernel(x, wq, wk, wv, wo, bo):
    global LAST_RESULT
    x = np.asarray(x, np.float32)
    in_maps = make_core_inputs(
        x, np.asarray(wq, np.float32), np.asarray(wk, np.float32),
        np.asarray(wv, np.float32), np.asarray(wo, np.float32),
        np.asarray(bo, np.float32))

    nc = get_nc()
    trace = bool(int(os.environ.get("KERNEL_TRACE", "0")))
    kwargs = {}
    if trace:
        kwargs.update(trace=True, trace_cores=[0, 1],
                      tmpdir=os.environ.get("KERNEL_TRACE_DIR") or None)
    res = run_bass_kernel_spmd(nc, in_maps, list(range(N_CORES)), **kwargs)
    LAST_RESULT = res

    out = np.empty((B, S, D), np.float32)
    for c in range(N_CORES):
        b, half = c // 2, c % 2
        out[b, half::2, :] = res.results[c]["out"]
    return out



# revision 7
# speedup vs baseline: 1.7112x; 1.7112x over previous
"""Trainium2 Bass kernel for 12-head causal MHA (B=4, S=2048, D=768).

Sharding: 8 cores, core c -> (batch c//2, head-half c%2).  Each core
computes 6 heads over ALL 2048 queries of its batch and emits the
PARTIAL out-projection (its 384 ctx dims x woT slice); the host sums
the two half-partials per batch and adds the bias.  This removes the
K/V-projection duplication of batch x query-parity sharding and makes
queries contiguous (simple causal masks).

Layout is fully transposed so every matmul contracts along partitions:
  qT/kT: [head_dim, seq]  scoresT: [sk, sq]  ctxT: [hd+1, sq]
The softmax row-sum is fused into the ctx matmul via a ones column
appended to V (M=65).  Softmax skips max-subtraction (scores/8 are
bounded by ~2 for this distribution, exp is safe).

Schedule: projections (512-key groups), attention blocks (256 queries)
and the out-projection are interleaved in one instruction stream so the
PE never idles long enough to drop out of its max p-state.  The
attention inner loop is software-pipelined (ctx of pair p issues after
scores of pair p+1, so exp/mask latency is hidden), and softmax
normalization (reciprocal -> gpsimd partition-broadcast -> scale) runs
entirely off the tensor engine, deferred into the next stream.
"""

import os
import sys
from contextlib import ExitStack

import numpy as np

os.environ.setdefault("MYCRO_LOCAL_CACHE", "1")

for _p in ("/root/.axon_site/_ro/trn_rl_repo", "/opt/trn_rl_repo"):
    # later inserts win: prefer /opt (writable sibling modules, e.g.
    # antenv.axon_hooks) over the read-only mirror
    if os.path.isdir(_p) and _p not in sys.path:
        sys.path.insert(0, _p)

import concourse.bass as bass  # noqa: E402
import concourse.tile as tile  # noqa: E402
from concourse import bacc, mybir  # noqa: E402
from concourse.bass_utils import run_bass_kernel_spmd  # noqa: E402

B, S, D, H, HD = 4, 2048, 768, 12, 64
HH = H // 2             # 6 heads per core
DH = HH * HD            # 384 ctx dims per core
NPAIR = HH // 2         # 3 head pairs (2 heads packed per 128 partitions)
KC = S // 128           # 16 key chunks
DC = D // 128           # 6 contraction chunks for the projections
NJ = S // 256           # 8 query blocks of 256
NG = 4                  # 4 groups of 512 keys/queries for the projections
N_CORES = 8

F32 = mybir.dt.float32
BF16 = mybir.dt.bfloat16
EXP = mybir.ActivationFunctionType.Exp

LAST_RESULT = None  # BassKernelResults of the most recent run (for test.py)

_CACHED_NC = None


def build_nc():
    nc = bacc.Bacc("TRN2", target_bir_lowering=False)

    xT = nc.dram_tensor("xT", [D, S], BF16, kind="ExternalInput")
    wqT = nc.dram_tensor("wqT", [D, DH], BF16, kind="ExternalInput")
    wkT = nc.dram_tensor("wkT", [D, DH], BF16, kind="ExternalInput")
    wvT = nc.dram_tensor("wvT", [D, DH], BF16, kind="ExternalInput")
    woT = nc.dram_tensor("woT", [DH, D], BF16, kind="ExternalInput")
    tri_d = nc.dram_tensor("tri", [128, 128], BF16, kind="ExternalInput")
    out_d = nc.dram_tensor("out", [S, D], F32, kind="ExternalOutput")

    with tile.TileContext(nc) as tc, ExitStack() as ctx:
        pers = ctx.enter_context(tc.tile_pool(name="pers", bufs=1))
        kT3 = pers.tile([128, NPAIR, S], BF16)          # kT, pair-stacked
        qT3 = pers.tile([128, NPAIR, S], BF16)
        v3 = pers.tile([128, KC, HH, HD + 1], BF16)     # v (+ones col) per chunk
        ctx3 = pers.tile([128, NPAIR, S], BF16)         # normalized ctxT
        tri = pers.tile([128, 128], BF16)               # causal k<=u mask
        ones_bf = pers.tile([128, 128], BF16)           # bcast matmul lhsT
        wq_sb = pers.tile([128, DC, DH], BF16)
        wk_sb = pers.tile([128, DC, DH], BF16)
        wv_sb = pers.tile([128, DC, DH], BF16)
        wo_sb = pers.tile([128, NPAIR, D], BF16)

        work = ctx.enter_context(tc.tile_pool(name="work", bufs=1))
        spool = ctx.enter_context(tc.tile_pool(name="spool", bufs=1, space="PSUM"))

        nc.vector.memset(v3[:, :, :, HD], 1.0)          # ones cols, stride 65
        nc.vector.memset(ones_bf, 1.0)
        nc.sync.dma_start(out=tri, in_=tri_d[:])
        for k in range(DC):
            nc.sync.dma_start(out=wq_sb[:, k, :], in_=wqT[128 * k:128 * (k + 1), :])
            nc.sync.dma_start(out=wk_sb[:, k, :], in_=wkT[128 * k:128 * (k + 1), :])
            nc.sync.dma_start(out=wv_sb[:, k, :], in_=wvT[128 * k:128 * (k + 1), :])
        for r in range(NPAIR):
            nc.sync.dma_start(out=wo_sb[:, r, :], in_=woT[128 * r:128 * (r + 1), :])

        pending_norm = []

        def normalize(r, j, cab):
            """Drain one head-pair/query-block: bf16-cast the fused row-sums,
            broadcast them across partitions with a rank-1 bf16 matmul,
            reciprocal the full tile (approx is exact enough), scale, and
            remap head B to partitions 64-127 via SBUF DMA."""
            jsl = slice(256 * j, 256 * (j + 1))
            rr = work.tile([65, 512], BF16, tag="rr", bufs=2, name="rr")
            nc.vector.tensor_copy(rr[64:65, :], cab[64:65, :])
            pb = spool.tile([128, 512], F32, tag="p", bufs=2, name="pb")
            nc.tensor.matmul(pb, lhsT=ones_bf[64:65, :], rhs=rr[64:65, :],
                             start=True, stop=True)
            pbr = work.tile([128, 512], F32, tag="pbr", bufs=2, name="pbr")
            nc.vector.reciprocal_approx_fast(pbr, pb)
            nc.vector.tensor_mul(ctx3[0:64, r, jsl], cab[0:64, 0:256],
                                 pbr[0:64, 0:256])
            tB = work.tile([64, 256], BF16, tag="tB", bufs=2, name="tB")
            nc.vector.tensor_mul(tB, cab[0:64, 256:512], pbr[0:64, 256:512])
            nc.sync.dma_start(out=ctx3[64:128, r, jsl], in_=tB)

        def flush_norm():
            while pending_norm:
                r, j, cab = pending_norm.pop(0)
                normalize(r, j, cab)

        def attn_block(j):
            jsl = slice(256 * j, 256 * (j + 1))
            npairs = j + 1
            for r in range(NPAIR):
                cab = spool.tile([65, 512], F32, tag="cab", bufs=2, name="cab")
                e_tiles = {}

                def scores(p):
                    sp = spool.tile([128, 1024], F32, tag="s", bufs=2, name="sp")
                    diag = p == j
                    for si in range(2):
                        a = 2 * p + si
                        asl = slice(128 * a, 128 * (a + 1))
                        zs = 128 if (diag and si == 1) else 0
                        qsl = slice(256 * j + zs, 256 * (j + 1))
                        # bank layout: [0:512) head-A scores of sites 2p,2p+1
                        # (bank 0); [512:1024) head-B (bank 1).  start=True
                        # clears the whole bank, so only the first matmul per
                        # bank sets it; the second lands as a fresh-element
                        # overwrite with start=False.
                        nc.tensor.matmul(
                            sp[:, 256 * si + zs:256 * (si + 1)],
                            lhsT=kT3[0:64, r, asl], rhs=qT3[0:64, r, qsl],
                            start=(si == 0), stop=True,
                            tile_position=(0, 0), skip_group_check=True)
                        nc.tensor.matmul(
                            sp[:, 512 + 256 * si + zs:512 + 256 * (si + 1)],
                            lhsT=kT3[64:128, r, asl], rhs=qT3[64:128, r, qsl],
                            start=(si == 0), stop=True,
                            tile_position=(64, 0), skip_group_check=True)
                    e = work.tile([128, 1024], BF16, tag="e", bufs=3, name="e")
                    nc.scalar.activation(e, sp, EXP, scale=0.125)
                    e_tiles[p] = e
                    if diag:
                        # partial strips of the two diagonal sites; one
                        # k<=u triangle serves all four.  On the (otherwise
                        # idle) pool engine so the DVE queue never delays
                        # the dependent ctx matmuls.
                        for off in (0, 384, 512, 896):
                            nc.gpsimd.tensor_mul(
                                e[:, off:off + 128], e[:, off:off + 128], tri)

                def ctxmm(p):
                    e = e_tiles.pop(p)
                    diag = p == j
                    for si in range(2):
                        a = 2 * p + si
                        zc = 128 if (diag and si == 1) else 0
                        st = (a == 0)
                        sto = (a == 2 * j + 1)
                        nc.tensor.matmul(
                            cab[0:65, zc:256], lhsT=v3[:, a, 2 * r, :],
                            rhs=e[:, 256 * si + zc:256 * (si + 1)],
                            start=st, stop=sto, skip_group_check=True)
                        nc.tensor.matmul(
                            cab[0:65, 256 + zc:512], lhsT=v3[:, a, 2 * r + 1, :],
                            rhs=e[:, 512 + 256 * si + zc:512 + 256 * (si + 1)],
                            start=False, stop=sto, skip_group_check=True)

                scores(0)
                flush_norm()   # previous stream's softmax drain, off-PE
                for p in range(1, npairs):
                    scores(p)
                    ctxmm(p - 1)
                ctxmm(npairs - 1)
                pending_norm.append((r, j, cab))

        def out_block(j):
            for i in (2 * j, 2 * j + 1):
                isl = slice(128 * i, 128 * (i + 1))
                for lo in (0, DH):
                    po = spool.tile([128, 512], F32, tag="p", bufs=2, name="po")
                    for r in range(NPAIR):
                        nc.tensor.matmul(
                            po[:, 0:DH], lhsT=ctx3[:, r, isl],
                            rhs=wo_sb[:, r, lo:lo + DH],
                            start=(r == 0), stop=(r == NPAIR - 1))
                    osb = work.tile([128, DH], F32, tag="osb", bufs=3, name="osb")
                    nc.vector.tensor_copy(osb, po[:, 0:DH])
                    nc.sync.dma_start(out=out_d[isl, lo:lo + DH], in_=osb)

        for g in range(NG):
            gsl = slice(512 * g, 512 * (g + 1))
            x_sb = work.tile([128, DC, 512], BF16, tag="x", bufs=2, name="x_sb")
            for k in range(DC):
                nc.sync.dma_start(
                    out=x_sb[:, k, :], in_=xT[128 * k:128 * (k + 1), gsl])
            # K projection for keys [512g, 512g+512)
            for r in range(NPAIR):
                ps = spool.tile([128, 512], F32, tag="p", bufs=2, name="psk")
                for k in range(DC):
                    nc.tensor.matmul(
                        ps, lhsT=wk_sb[:, k, 128 * r:128 * (r + 1)],
                        rhs=x_sb[:, k, :], start=(k == 0), stop=(k == DC - 1))
                nc.vector.tensor_copy(kT3[:, r, gsl], ps)
            # V projection per 128-key chunk
            for aa in range(4):
                a = 4 * g + aa
                ps = spool.tile([128, 512], F32, tag="p", bufs=2, name="psv")
                for k in range(DC):
                    nc.tensor.matmul(
                        ps[:, 0:DH], lhsT=x_sb[:, k, 128 * aa:128 * (aa + 1)],
                        rhs=wv_sb[:, k, :], start=(k == 0), stop=(k == DC - 1))
                nc.vector.tensor_copy(
                    v3[:, a, :, 0:HD],
                    ps[:, 0:DH].rearrange("p (h e) -> p h e", e=HD))
            # Q projection for queries [512g, 512g+512)
            for r in range(NPAIR):
                ps = spool.tile([128, 512], F32, tag="p", bufs=2, name="psq")
                for k in range(DC):
                    nc.tensor.matmul(
                        ps, lhsT=wq_sb[:, k, 128 * r:128 * (r + 1)],
                        rhs=x_sb[:, k, :], start=(k == 0), stop=(k == DC - 1))
                nc.vector.tensor_copy(qT3[:, r, gsl], ps)

            attn_block(2 * g)
            if g > 0:
                out_block(2 * g - 1)
            attn_block(2 * g + 1)
            out_block(2 * g)

        flush_norm()
        out_block(NJ - 1)

    nc.compile()
    return nc


def get_nc():
    global _CACHED_NC
    if _CACHED_NC is None:
        _CACHED_NC = build_nc()
    return _CACHED_NC


def make_core_inputs(x, wq, wk, wv, wo):
    """Host-side shard prep: slices/transposes/dtype rounding only."""
    import ml_dtypes
    bf16 = ml_dtypes.bfloat16

    tri = (np.arange(128)[:, None] <= np.arange(128)[None, :]).astype(bf16)

    wslices = []
    for hh in range(2):
        hsl = slice(DH * hh, DH * (hh + 1))
        wslices.append({
            "wqT": np.ascontiguousarray(wq[hsl, :].T.astype(bf16)),
            "wkT": np.ascontiguousarray(wk[hsl, :].T.astype(bf16)),
            "wvT": np.ascontiguousarray(wv[hsl, :].T.astype(bf16)),
            "woT": np.ascontiguousarray(wo[:, hsl].T.astype(bf16)),
        })

    in_maps = []
    for c in range(N_CORES):
        b, hh = c // 2, c % 2
        xT_b = np.ascontiguousarray(x[b].T.astype(bf16))
        m = {"xT": xT_b, "tri": tri}
        m.update(wslices[hh])
        in_maps.append(m)
    return in_maps


def kernel(x, wq, wk, wv, wo, bo):
    global LAST_RESULT
    x = np.asarray(x, np.float32)
    bo = np.asarray(bo, np.float32)
    in_maps = make_core_inputs(
        x, np.asarray(wq, np.float32), np.asarray(wk, np.float32),
        np.asarray(wv, np.float32), np.asarray(wo, np.float32))

    nc = get_nc()
    trace = bool(int(os.environ.get("KERNEL_TRACE", "0")))
    kwargs = {}
    if trace:
        kwargs.update(trace=True, trace_cores=[0, 1],
                      tmpdir=os.environ.get("KERNEL_TRACE_DIR") or None)
    res = run_bass_kernel_spmd(nc, in_maps, list(range(N_CORES)), **kwargs)
    LAST_RESULT = res

    out = np.empty((B, S, D), np.float32)
    for b in range(B):
        out[b] = res.results[2 * b]["out"] + res.results[2 * b + 1]["out"] + bo
    return out
